# revision 27
# baseline (speedup 1.0000x reference)
"""Trainium2 Bass kernel for nn_CLS_1889785610440.

Pipeline (per reference.py):
  3 scalar Elman RNNs over T in {4,8,16} for N=B*M*E lanes -> last hidden
  -> 1x3 conv over scales -> scalar RNN over M=64 -> BatchNorm1d (batch
  stats) -> ReLU -> Linear(E,C) -> softmax.

Key optimizations vs the v1 baseline:
  * Truncation-aware LOADING: the rnn2 recurrence over m contracts by
    |whh2| (=0.61) per step, so only the last Km ~ 12 of 64 m-positions
    influence the output above ~3e-3; stage-1 recurrences likewise only
    need their last Kt_s steps (|whh_s|^Kt_s <= 1e-3).  Lanes with
    m < M-Km and time-steps t < T-Kt_s are never uploaded or computed.
    (measured end-to-end error 2.2e-3 vs the 2e-2 gate)
  * bf16 upload: inputs are quantized host-side to bf16 (recurrence
    arithmetic stays fp32 on-chip), halving DMA bytes.
  * Host-side gather picks the lane order so stage-1 partitions are
    e_lo directly: free dim = (m, b_loc, e_hi).  The conv output IS the
    rnn2 input buffer - no PE transposes / scatters at all.
  * Stage-1 step on DVE+ACT: st = h*(whh/wih) + x_t (one
    scalar_tensor_tensor), h = tanh(wih*st + b) (one activation with
    folded scale+bias).  No matmuls, PSUM untouched until the FC.
  * Conv + rnn2 input affine folded into 2 DVE ops (pivot scale) and
    the rnn2 activation scale/bias.
  * BN normalize+relu fused into one ACT op per e_hi via per-partition
    scale/bias (relu(scl*f + shf)).
  * Single ACT table switch (Tanh -> Ln/Exp set) for the whole kernel.

Sharding: data-parallel over B=128 -> 16 samples/core.  Only the
BatchNorm statistics cross cores (one 2KB AllReduce).
"""

import math

import numpy as np
import ml_dtypes

import concourse.bacc as bacc
import concourse.tile as tile
import concourse.mybir as mybir
from concourse.bass_utils import run_bass_kernel_spmd

# Problem constants (hardcoded per spec).
B = 128
E = 256
M = 64
S = 3
C = 5
SCALES = [4, 8, 16]
EPS = 1e-5

N_CORES = 8
BLOC = B // N_CORES        # 16 samples per core
L2 = BLOC * 2              # 32 rnn2 lanes per free column group (b, e_hi)

FP32 = mybir.dt.float32
BF16 = mybir.dt.bfloat16
AF = mybir.ActivationFunctionType
ALU = mybir.AluOpType

# truncation tolerances (|w|^K bounds; end-to-end error is ~30x smaller
# because tanh' < 1 contracts further — measured 2.6e-3 total vs the
# 2e-2 gate at these settings)
TOL_STAGE1 = 1.5e-3
TOL_RNN2 = 7.5e-3


def _trunc_steps(aw, T, tol):
    if aw < 1e-12:
        return 1
    if aw >= 1.0:
        return T
    return min(T, max(1, int(math.ceil(math.log(tol) / math.log(aw)))))


def _plan(params):
    """Derived scalars: truncation depths, folded coefficients."""
    p = {}
    p["Kt"] = [_trunc_steps(abs(params["whh"][s]), SCALES[s], TOL_STAGE1)
               for s in range(S)]
    p["Km"] = _trunc_steps(abs(params["whh2"]), M, TOL_RNN2)
    # conv folded: u = sum_s c_s h_s + D with c_s = wih2*cw_s,
    # D = wih2*cb + bih2 + bhh2.  Chain through pivot scale pv
    # (largest |c_s|): t2 = sum_s (c_s/c_pv) h_s; rnn2 activation is
    # tanh(c_pv * (h*whh2/c_pv + t2_m) + D).
    c = [params["wih2"] * params["cw"][s] for s in range(S)]
    pv = int(np.argmax([abs(x) for x in c]))
    p["c"] = c
    p["pv"] = pv
    p["D"] = params["wih2"] * params["cb"] + params["bb2"]
    return p


def _build(params, n_devices=N_CORES, km_override=None, kt_override=None):
    nc = bacc.Bacc("TRN2", target_bir_lowering=False, debug=False,
                   enable_asserts=True, num_devices=n_devices)

    plan = _plan(params)
    Km = km_override or plan["Km"]
    Kt = kt_override or plan["Kt"]
    FC = Km * L2               # stage-1 free width: (m, b_loc, e_hi)
    c_s, D = plan["c"], plan["D"]
    wih, whh, bb = params["wih"], params["whh"], params["bb"]
    whh2 = params["whh2"]

    # t-block split per scale so the recurrence can start before the
    # whole x tensor lands: blocks of [1, 2, rest] steps.
    tblocks = []
    for s in range(S):
        blks, rem = [], Kt[s]
        for want in (1, 2):
            if rem > want:
                blks.append(want)
                rem -= want
        blks.append(rem)
        tblocks.append(blks)

    a_dram = [
        nc.dram_tensor(f"a{i}", [128 * FC * Kt[i]], BF16, kind="ExternalInput")
        for i in range(S)
    ]
    out_dram = nc.dram_tensor("out", [C, BLOC], FP32, kind="ExternalOutput")

    # One packed constant tensor:
    # [bias(5) | gb(4) | wpack(2C) | fnnb(1) | ones(6)]
    NCOL = 5 + 4 + 2 * C + 1 + 6
    cpack = np.zeros((128, NCOL), np.float32)
    cpack[:, 0] = bb[0]
    cpack[:, 1] = bb[1]
    cpack[:, 2] = bb[2]
    cpack[:, 3] = D
    cpack[:, 4] = EPS
    cpack[:, 5:7] = params["gamma"].reshape(2, 128).T
    cpack[:, 7:9] = params["beta"].reshape(2, 128).T
    fw = params["fnn_w"]  # (C, E); e = eh*128 + e_lo
    cpack[:, 9:9 + C] = fw[:, :128].T
    cpack[:, 9 + C:9 + 2 * C] = fw[:, 128:].T
    cpack[0:C, 9 + 2 * C] = params["fnn_b"]
    cpack[0:C, 10 + 2 * C:16 + 2 * C] = 1.0
    cpack_c = nc.inline_tensor(cpack, name="cpack")

    from contextlib import ExitStack
    with tile.TileContext(nc) as tc, ExitStack() as ctx:
        singles = ctx.enter_context(tc.tile_pool(name="singles", bufs=1))
        xp = [ctx.enter_context(tc.tile_pool(name=f"x{s}", bufs=1))
              for s in range(S)]
        hp = ctx.enter_context(tc.tile_pool(name="h", bufs=6))
        stp = ctx.enter_context(tc.tile_pool(name="st", bufs=4))
        cvp = ctx.enter_context(tc.tile_pool(name="cv", bufs=2))
        r2p = ctx.enter_context(tc.tile_pool(name="r2", bufs=4))
        smp = ctx.enter_context(tc.tile_pool(name="sm", bufs=2))
        pst = ctx.enter_context(tc.tile_pool(name="pst", bufs=1, space="PSUM"))
        dram = ctx.enter_context(tc.tile_pool(name="dram", bufs=1, space="DRAM"))

        consts = singles.tile([128, NCOL], FP32)
        nc.sync.dma_start(out=consts[:], in_=cpack_c[:])
        bias_c = consts[:, 0:5]
        gb = consts[:, 5:9]
        wpack = consts[:, 9:9 + 2 * C]
        fnnb = consts[0:C, 9 + 2 * C:10 + 2 * C]
        ones_col = consts[0:C, 10 + 2 * C:11 + 2 * C]
        ones_row = consts[0:1, 10 + 2 * C:15 + 2 * C]

        # x tiles, one per (scale, t-block), t-major so each step's
        # slice is contiguous.  DMAs are spread over several engine
        # rings so their issue doesn't serialize, ordered so the first
        # steps of every scale land early.
        xt = [[None] * len(tblocks[s]) for s in range(S)]
        av = [a_dram[s].ap().rearrange("(p ft) -> p ft", p=128)
              for s in range(S)]
        # one ring per scale so every scale's first block issues
        # immediately (consts share the sync ring ahead of scale 0)
        rings = [nc.sync, nc.scalar, nc.gpsimd]
        order = [(s, j) for j in range(3) for s in range(S)
                 if j < len(tblocks[s])]
        for s, j in order:
            tb = tblocks[s]
            t_lo = sum(tb[:j])
            x = xp[s].tile([128, tb[j], FC], BF16, tag=f"x{s}_{j}",
                           name=f"x{s}_{j}")
            rings[s].dma_start(
                out=x[:].rearrange("p t f -> p (t f)"),
                in_=av[s][:, FC * t_lo:FC * (t_lo + tb[j])])
            xt[s][j] = x

        def xcol(s, r, c0=0, c1=None):
            """x slice for global step r of scale s (contiguous)."""
            lo = 0
            for j, tb in enumerate(tblocks[s]):
                if r < lo + tb:
                    return xt[s][j][:, r - lo, c0:(FC if c1 is None else c1)]
                lo += tb
            raise AssertionError

        # ---- stage-1: 3 interleaved recurrences over t, with the
        # conv partial fused in as soon as two scales complete ----
        # conv folded to pivot: t2 = sum_s (c_s/c_pv) h_s; the scale
        # finishing LAST is chained in last.
        fin_round = sorted(range(S), key=lambda s: (Kt[s], s))
        early2, last1 = fin_round[:2], fin_round[2]
        cmax = max(abs(x) for x in c_s)
        if (abs(c_s[early2[1]]) < 1e-6 * cmax
                or abs(c_s[last1]) < 1e-6 * cmax):
            # degenerate conv weights: chain in increasing-|c| order so
            # every ratio is <= ~1 (loses the early-combine overlap)
            by_mag = sorted(range(S), key=lambda s: abs(c_s[s]))
            early2, last1 = by_mag[:2], by_mag[2]
        h_cur = [None] * S
        t1 = cvp.tile([128, FC], FP32, tag="t1", name="t1")
        t2 = cvp.tile([128, FC], FP32, tag="t2", name="t2")
        # once only one scale remains, its dependent chain is the only
        # work: split it into column halves so two chains pipeline
        # (roughly halves the latency-bound stretch)
        lone_start = max(Kt[s] for s in range(S) if s != last1)
        H = FC // 2
        h_half = [None, None]
        for r in range(max(Kt)):
            for s in range(S):
                if r >= Kt[s]:
                    continue
                if s == last1 and r >= lone_start:
                    for i in range(2):
                        lo, hi = i * H, (i + 1) * H
                        src = (h_half[i][:] if h_half[i] is not None
                               else h_cur[s][:, lo:hi])
                        hh = hp.tile([128, H], FP32, tag=f"hh{i}",
                                     name=f"hh{i}")
                        sth = stp.tile([128, H], FP32, tag=f"sth{i}",
                                       name=f"sth{i}")
                        nc.vector.scalar_tensor_tensor(
                            sth[:], src, whh[s] / wih[s],
                            xcol(s, r, lo, hi), op0=ALU.mult, op1=ALU.add)
                        nc.scalar.activation(hh[:], sth[:], AF.Tanh,
                                             bias=bias_c[:, s:s + 1],
                                             scale=wih[s])
                        h_half[i] = hh
                    continue
                hn = hp.tile([128, FC], FP32, tag=f"h{s}", name=f"h{s}")
                if h_cur[s] is None:
                    nc.scalar.activation(hn[:], xcol(s, r), AF.Tanh,
                                         bias=bias_c[:, s:s + 1],
                                         scale=wih[s])
                else:
                    st = stp.tile([128, FC], FP32, tag=f"st{s}",
                                  name=f"st{s}")
                    nc.vector.scalar_tensor_tensor(
                        st[:], h_cur[s][:], whh[s] / wih[s], xcol(s, r),
                        op0=ALU.mult, op1=ALU.add)
                    nc.scalar.activation(hn[:], st[:], AF.Tanh,
                                         bias=bias_c[:, s:s + 1],
                                         scale=wih[s])
                h_cur[s] = hn
            if r == max(Kt[early2[0]], Kt[early2[1]]) - 1:
                # both early scales done: combine them now (overlaps
                # the remaining lone-scale rounds)
                a, b = early2
                nc.vector.scalar_tensor_tensor(
                    t1[:], h_cur[a][:], c_s[a] / c_s[b], h_cur[b][:],
                    op0=ALU.mult, op1=ALU.add)
        rr = c_s[early2[1]] / c_s[last1]
        if h_half[0] is not None:
            for i in range(2):
                lo, hi = i * H, (i + 1) * H
                nc.vector.scalar_tensor_tensor(
                    t2[:, lo:hi], t1[:, lo:hi], rr, h_half[i][:],
                    op0=ALU.mult, op1=ALU.add)
        else:
            nc.vector.scalar_tensor_tensor(
                t2[:], t1[:], rr, h_cur[last1][:],
                op0=ALU.mult, op1=ALU.add)
        # t2 = sum_s (c_s/c_last) h_s; rnn2 activation scale is c_last
        cpv = c_s[last1]
        u2 = t2[:].rearrange("p (m l) -> p m l", m=Km)

        # ---- rnn2 over m (chain; only last hidden needed) ----
        h2 = None
        feat = smp.tile([128, L2], FP32, tag="feat", name="feat")
        for m in range(Km):
            last = m == Km - 1
            dst = feat[:] if last else r2p.tile(
                [128, L2], FP32, tag="h2", name="h2")[:]
            if h2 is None:
                nc.scalar.activation(dst, u2[:, m, :], AF.Tanh,
                                     bias=bias_c[:, 3:4], scale=cpv)
            else:
                st2 = r2p.tile([128, L2], FP32, tag="st2", name="st2")
                nc.vector.scalar_tensor_tensor(
                    st2[:], h2, whh2 / cpv, u2[:, m, :],
                    op0=ALU.mult, op1=ALU.add)
                nc.scalar.activation(dst, st2[:], AF.Tanh,
                                     bias=bias_c[:, 3:4], scale=cpv)
            h2 = dst

        # ---- BatchNorm stats (partial): sums over local b ----
        featsq = smp.tile([128, L2], FP32, tag="fsq", name="fsq")
        nc.vector.tensor_tensor(featsq[:], feat[:], feat[:], ALU.mult)
        stats = smp.tile([128, 4], FP32, tag="stats", name="stats")
        fv = feat[:].rearrange("p (b eh) -> p eh b", b=BLOC)
        fsv = featsq[:].rearrange("p (b eh) -> p eh b", b=BLOC)
        nc.vector.tensor_reduce(stats[:, 0:2], fv,
                                axis=mybir.AxisListType.X, op=ALU.add)
        nc.vector.tensor_reduce(stats[:, 2:4], fsv,
                                axis=mybir.AxisListType.X, op=ALU.add)

        # AllGather the per-core partial sums (latency floor ~2x lower
        # than AllReduce for this 2KB message), reduce locally.
        bin_ = dram.tile([128, 4], FP32, tag="bin")
        bout = dram.tile([N_CORES * 128, 4], FP32, tag="bout")
        nc.sync.dma_start(bin_[:], stats[:])
        nc.gpsimd.collective_compute(
            "AllGather", ALU.bypass,
            replica_groups=[list(range(N_CORES))],
            ins=[bin_.opt()], outs=[bout.opt()])
        stg8 = smp.tile([128, N_CORES, 4], FP32, tag="stg8")
        bv = bout[:].rearrange("(g p) f -> p g f", p=128)
        half = N_CORES // 2
        nc.sync.dma_start(stg8[:, 0:half, :], bv[:, 0:half, :])
        nc.scalar.dma_start(stg8[:, half:, :], bv[:, half:, :])
        stg = smp.tile([128, 4], FP32, tag="stg")
        nc.vector.tensor_reduce(
            stg[:], stg8[:].rearrange("p g f -> p f g"),
            axis=mybir.AxisListType.X, op=ALU.add)

        # ---- BN scale/shift: istd = exp(-0.5 ln(var+eps)) ----
        mex = smp.tile([128, 4], FP32, tag="mex")
        # cols 0:2 = mean; cols 2:4 = E[x^2] + eps (eps folded here)
        nc.vector.tensor_scalar(mex[:, 0:2], stg[:, 0:2], 1.0 / B, None,
                                ALU.mult)
        nc.vector.tensor_scalar(mex[:, 2:4], stg[:, 2:4], 1.0 / B, EPS,
                                ALU.mult, op1=ALU.add)
        mean = mex[:, 0:2]
        var = smp.tile([128, 2], FP32, tag="var")
        nc.vector.tensor_tensor(var[:], mean, mean, ALU.mult)
        nc.vector.tensor_tensor(var[:], mex[:, 2:4], var[:], ALU.subtract)
        lnv = smp.tile([128, 2], FP32, tag="lnv")
        nc.scalar.activation(lnv[:], var[:], AF.Ln)
        istd = smp.tile([128, 2], FP32, tag="istd")
        nc.scalar.activation(istd[:], lnv[:], AF.Exp, scale=-0.5)
        scl = smp.tile([128, 2], FP32, tag="scl")
        nc.vector.tensor_tensor(scl[:], istd[:], gb[:, 0:2], ALU.mult)
        shf = smp.tile([128, 2], FP32, tag="shf")
        nc.vector.tensor_tensor(shf[:], mean, scl[:], ALU.mult)
        nc.vector.tensor_tensor(shf[:], gb[:, 2:4], shf[:], ALU.subtract)

        # ---- normalize + relu fused: relu(scl*f + shf) per e_hi ----
        r = smp.tile([128, L2], FP32, tag="r")
        f3 = feat[:].rearrange("p (b eh) -> p b eh", b=BLOC)
        r3 = r[:].rearrange("p (b eh) -> p b eh", b=BLOC)
        for eh in range(2):
            nc.scalar.activation(r3[:, :, eh], f3[:, :, eh], AF.Relu,
                                 bias=shf[:, eh:eh + 1],
                                 scale=scl[:, eh:eh + 1])

        # ---- FC: logits^T (C, BLOC) = sum_eh Wpack_eh^T @ r[:, :, eh] ----
        tailps = pst.tile([128, 512], FP32, tag="tailps")
        pl = tailps[0:C, 0:BLOC]
        nc.tensor.matmul(pl, wpack[:, 0:C], r3[:, :, 0],
                         start=True, stop=False)
        nc.tensor.matmul(pl, wpack[:, C:2 * C], r3[:, :, 1],
                         start=False, stop=True)

        # ---- softmax directly on the (C, BLOC) layout.  Logits are
        # O(10) here (normalized features through W/C weights), so the
        # max-subtraction is unnecessary in fp32.  Sum over classes via
        # a tiny matmul; the host transposes the (C, BLOC) output. ----
        esb = smp.tile([C, BLOC], FP32, tag="esb")
        nc.scalar.activation(esb[:], pl, AF.Exp, bias=fnnb[:, 0:1])
        psum_s = tailps[0:1, 64:64 + BLOC]
        nc.tensor.matmul(psum_s, ones_col, esb[:], start=True, stop=True)
        rin = smp.tile([1, BLOC], FP32, tag="rin")
        nc.vector.reciprocal(rin[:], psum_s)
        pbc = tailps[0:C, 96:96 + BLOC]
        nc.tensor.matmul(pbc, ones_row, rin[:], start=True, stop=True)
        osb = smp.tile([C, BLOC], FP32, tag="osb")
        nc.vector.tensor_tensor(osb[:], esb[:], pbc, ALU.mult)
        nc.sync.dma_start(out=out_dram[:], in_=osb[:])

    nc.compile()
    return nc, Km, Kt, [list(tb) for tb in
                        ([tblocks[s] for s in range(S)])]


def _gather_core(a_list, k, Km, Kt, tblocks):
    """Host-side gather for core k: bf16, layout [e_lo, tblock, t, (m b eh)]."""
    out = []
    for s in range(S):
        T = SCALES[s]
        A = np.asarray(a_list[s])[:, :, 0].reshape(B, M, 2, 128, T)
        Sv = A[k * BLOC:(k + 1) * BLOC, M - Km:, :, :, T - Kt[s]:]
        # [b, m, eh, e_lo, t] -> [e_lo, t, m, b, eh]
        Sv = np.transpose(Sv, (3, 4, 1, 0, 2))
        parts = []
        t_lo = 0
        for tb in tblocks[s]:
            blk = Sv[:, t_lo:t_lo + tb]
            parts.append(np.ascontiguousarray(blk).reshape(128, -1))
            t_lo += tb
        full = np.concatenate(parts, axis=1)
        out.append(full.astype(ml_dtypes.bfloat16).reshape(-1))
    return out


def kernel(a0, a1, a2, rnn1_wih, rnn1_whh, rnn1_bih, rnn1_bhh,
           conv_w, conv_b, rnn2_wih, rnn2_whh, rnn2_bih, rnn2_bhh,
           norm_gamma, norm_beta, fnn_w, fnn_b, _bench=None,
           _km=None, _kt=None):
    params = {
        "wih": [float(rnn1_wih[s]) for s in range(S)],
        "whh": [float(rnn1_whh[s]) for s in range(S)],
        "bb": [float(rnn1_bih[s]) + float(rnn1_bhh[s]) for s in range(S)],
        "cw": [float(conv_w[s]) for s in range(S)],
        "cb": float(conv_b[0]),
        "wih2": float(rnn2_wih[0]),
        "whh2": float(rnn2_whh[0]),
        "bb2": float(rnn2_bih[0]) + float(rnn2_bhh[0]),
        "gamma": np.asarray(norm_gamma, np.float32),
        "beta": np.asarray(norm_beta, np.float32),
        "fnn_w": np.asarray(fnn_w, np.float32),
        "fnn_b": np.asarray(fnn_b, np.float32),
    }
    nc, Km, Kt, tblocks = _build(params, km_override=_km, kt_override=_kt)

    in_maps = []
    for k in range(N_CORES):
        arrs = _gather_core((a0, a1, a2), k, Km, Kt, tblocks)
        in_maps.append({f"a{i}": arrs[i] for i in range(S)})

    kw = dict(_bench) if _bench else {}
    res = run_bass_kernel_spmd(nc, in_maps, core_ids=list(range(N_CORES)),
                               **kw)
    out = np.concatenate([res.results[k]["out"].T for k in range(N_CORES)],
                         axis=0)
    if _bench is not None:
        kernel.last_result = res
    return out


# revision 34
# speedup vs baseline: 1.0636x; 1.0636x over previous
"""Trainium2 Bass kernel for nn_CLS_1889785610440.

Pipeline (per reference.py):
  3 scalar Elman RNNs over T in {4,8,16} for N=B*M*E lanes -> last hidden
  -> 1x3 conv over scales -> scalar RNN over M=64 -> BatchNorm1d (batch
  stats) -> ReLU -> Linear(E,C) -> softmax.

Key optimizations vs the v1 baseline:
  * Truncation-aware LOADING: the rnn2 recurrence over m contracts by
    |whh2| (=0.61) per step, so only the last Km ~ 12 of 64 m-positions
    influence the output above ~3e-3; stage-1 recurrences likewise only
    need their last Kt_s steps (|whh_s|^Kt_s <= 1e-3).  Lanes with
    m < M-Km and time-steps t < T-Kt_s are never uploaded or computed.
    (measured end-to-end error 2.2e-3 vs the 2e-2 gate)
  * bf16 upload: inputs are quantized host-side to bf16 (recurrence
    arithmetic stays fp32 on-chip), halving DMA bytes.
  * Host-side gather picks the lane order so stage-1 partitions are
    e_lo directly: free dim = (m, b_loc, e_hi).  The conv output IS the
    rnn2 input buffer - no PE transposes / scatters at all.
  * Stage-1 step on DVE+ACT: st = h*(whh/wih) + x_t (one
    scalar_tensor_tensor), h = tanh(wih*st + b) (one activation with
    folded scale+bias).  No matmuls, PSUM untouched until the FC.
  * Conv + rnn2 input affine folded into 2 DVE ops (pivot scale) and
    the rnn2 activation scale/bias.
  * BN normalize+relu fused into one ACT op per e_hi via per-partition
    scale/bias (relu(scl*f + shf)).
  * Single ACT table switch (Tanh -> Ln/Exp set) for the whole kernel.

Sharding: data-parallel over B=128 -> 16 samples/core.  Only the
BatchNorm statistics cross cores (one 2KB AllReduce).
"""

import math

import numpy as np
import ml_dtypes

import concourse.bacc as bacc
import concourse.tile as tile
import concourse.mybir as mybir
from concourse.bass_utils import run_bass_kernel_spmd

# Problem constants (hardcoded per spec).
B = 128
E = 256
M = 64
S = 3
C = 5
SCALES = [4, 8, 16]
EPS = 1e-5

N_CORES = 8
BLOC = B // N_CORES        # 16 samples per core
L2 = BLOC * 2              # 32 rnn2 lanes per free column group (b, e_hi)

FP32 = mybir.dt.float32
BF16 = mybir.dt.bfloat16
AF = mybir.ActivationFunctionType
ALU = mybir.AluOpType

# truncation tolerances (|w|^K bounds; end-to-end error is ~30x smaller
# because tanh' < 1 contracts further — measured 2.6e-3 total vs the
# 2e-2 gate at these settings)
TOL_STAGE1 = 1.5e-3
TOL_RNN2 = 7.5e-3


def _trunc_steps(aw, T, tol):
    if aw < 1e-12:
        return 1
    if aw >= 1.0:
        return T
    return min(T, max(1, int(math.ceil(math.log(tol) / math.log(aw)))))


def _plan(params):
    """Derived scalars: truncation depths, folded coefficients."""
    p = {}
    p["Kt"] = [_trunc_steps(abs(params["whh"][s]), SCALES[s], TOL_STAGE1)
               for s in range(S)]
    p["Km"] = _trunc_steps(abs(params["whh2"]), M, TOL_RNN2)
    # conv folded: u = sum_s c_s h_s + D with c_s = wih2*cw_s,
    # D = wih2*cb + bih2 + bhh2.  Chain through pivot scale pv
    # (largest |c_s|): t2 = sum_s (c_s/c_pv) h_s; rnn2 activation is
    # tanh(c_pv * (h*whh2/c_pv + t2_m) + D).
    c = [params["wih2"] * params["cw"][s] for s in range(S)]
    pv = int(np.argmax([abs(x) for x in c]))
    p["c"] = c
    p["pv"] = pv
    p["D"] = params["wih2"] * params["cb"] + params["bb2"]
    return p


def _build(params, n_devices=N_CORES, km_override=None, kt_override=None):
    nc = bacc.Bacc("TRN2", target_bir_lowering=False, debug=False,
                   enable_asserts=True, num_devices=n_devices)

    plan = _plan(params)
    Km = km_override or plan["Km"]
    Kt = kt_override or plan["Kt"]
    FC = Km * L2               # stage-1 free width: (m, b_loc, e_hi)
    c_s, D = plan["c"], plan["D"]
    wih, whh, bb = params["wih"], params["whh"], params["bb"]
    whh2 = params["whh2"]

    # t-block split per scale so the recurrence can start before the
    # whole x tensor lands: blocks of [1, 2, rest] steps.
    tblocks = []
    for s in range(S):
        blks, rem = [], Kt[s]
        for want in (1, 2):
            if rem > want:
                blks.append(want)
                rem -= want
        blks.append(rem)
        tblocks.append(blks)

    a_dram = [
        nc.dram_tensor(f"a{i}", [128 * FC * Kt[i]], BF16, kind="ExternalInput")
        for i in range(S)
    ]
    out_dram = nc.dram_tensor("out", [C, BLOC], FP32, kind="ExternalOutput")

    # One packed constant tensor:
    # [bias(5) | wpack(2C) | fnnb(1) | ones(6) | Mred(4) | eye4(4) |
    #  gamma_t(128) | beta_t(128)]
    NB = 5 + 2 * C + 1 + 6
    NCOL = NB + 4 + 4 + 256
    cpack = np.zeros((128, NCOL), np.float32)
    cpack[:, 0] = bb[0]
    cpack[:, 1] = bb[1]
    cpack[:, 2] = bb[2]
    cpack[:, 3] = D
    cpack[:, 4] = EPS
    fw = params["fnn_w"]  # (C, E); e = eh*128 + e_lo
    cpack[:, 5:5 + C] = fw[:, :128].T
    cpack[:, 5 + C:5 + 2 * C] = fw[:, 128:].T
    cpack[0:C, 5 + 2 * C] = params["fnn_b"]
    cpack[0:C, 6 + 2 * C:12 + 2 * C] = 1.0
    # Mred[q, f] = 1 where q%4 == f: partition-reduce of the gathered
    # [32,128] stats blocks down to [4,128] via one matmul
    q = np.arange(32)
    cpack[0:32, NB:NB + 4] = (q[:, None] % 4 == np.arange(4)[None, :])
    cpack[0:4, NB + 4:NB + 8] = np.eye(4, dtype=np.float32)
    cpack[0:2, NB + 8:NB + 136] = params["gamma"].reshape(2, 128)
    cpack[0:2, NB + 136:NB + 264] = params["beta"].reshape(2, 128)
    cpack_c = nc.inline_tensor(cpack, name="cpack")

    from contextlib import ExitStack
    with tile.TileContext(nc) as tc, ExitStack() as ctx:
        singles = ctx.enter_context(tc.tile_pool(name="singles", bufs=1))
        xp = [ctx.enter_context(tc.tile_pool(name=f"x{s}", bufs=1))
              for s in range(S)]
        hp = ctx.enter_context(tc.tile_pool(name="h", bufs=6))
        stp = ctx.enter_context(tc.tile_pool(name="st", bufs=4))
        cvp = ctx.enter_context(tc.tile_pool(name="cv", bufs=2))
        r2p = ctx.enter_context(tc.tile_pool(name="r2", bufs=4))
        smp = ctx.enter_context(tc.tile_pool(name="sm", bufs=2))
        pst = ctx.enter_context(tc.tile_pool(name="pst", bufs=1, space="PSUM"))
        dram = ctx.enter_context(tc.tile_pool(name="dram", bufs=1, space="DRAM"))

        consts = singles.tile([128, NCOL], FP32)
        nc.sync.dma_start(out=consts[:], in_=cpack_c[:])
        bias_c = consts[:, 0:5]
        wpack = consts[:, 5:5 + 2 * C]
        fnnb = consts[0:C, 5 + 2 * C:6 + 2 * C]
        ones_col = consts[0:C, 6 + 2 * C:7 + 2 * C]
        ones_row = consts[0:1, 6 + 2 * C:11 + 2 * C]
        mred = consts[0:32, NB:NB + 4]
        eye4 = consts[0:4, NB + 4:NB + 8]
        gamma_t = consts[0:2, NB + 8:NB + 136]
        beta_t = consts[0:2, NB + 136:NB + 264]

        # x tiles, one per (scale, t-block), t-major so each step's
        # slice is contiguous.  DMAs are spread over several engine
        # rings so their issue doesn't serialize, ordered so the first
        # steps of every scale land early.
        xt = [[None] * len(tblocks[s]) for s in range(S)]
        av = [a_dram[s].ap().rearrange("(p ft) -> p ft", p=128)
              for s in range(S)]
        # one ring per scale so every scale's first block issues
        # immediately (consts share the sync ring ahead of scale 0)
        rings = [nc.sync, nc.scalar, nc.gpsimd]
        order = [(s, j) for j in range(3) for s in range(S)
                 if j < len(tblocks[s])]
        for s, j in order:
            tb = tblocks[s]
            t_lo = sum(tb[:j])
            x = xp[s].tile([128, tb[j], FC], BF16, tag=f"x{s}_{j}",
                           name=f"x{s}_{j}")
            rings[s].dma_start(
                out=x[:].rearrange("p t f -> p (t f)"),
                in_=av[s][:, FC * t_lo:FC * (t_lo + tb[j])])
            xt[s][j] = x

        def xcol(s, r, c0=0, c1=None):
            """x slice for global step r of scale s (contiguous)."""
            lo = 0
            for j, tb in enumerate(tblocks[s]):
                if r < lo + tb:
                    return xt[s][j][:, r - lo, c0:(FC if c1 is None else c1)]
                lo += tb
            raise AssertionError

        # ---- stage-1: 3 interleaved recurrences over t, with the
        # conv partial fused in as soon as two scales complete ----
        # conv folded to pivot: t2 = sum_s (c_s/c_pv) h_s; the scale
        # finishing LAST is chained in last.
        fin_round = sorted(range(S), key=lambda s: (Kt[s], s))
        early2, last1 = fin_round[:2], fin_round[2]
        cmax = max(abs(x) for x in c_s)
        if (abs(c_s[early2[1]]) < 1e-6 * cmax
                or abs(c_s[last1]) < 1e-6 * cmax):
            # degenerate conv weights: chain in increasing-|c| order so
            # every ratio is <= ~1 (loses the early-combine overlap)
            by_mag = sorted(range(S), key=lambda s: abs(c_s[s]))
            early2, last1 = by_mag[:2], by_mag[2]
        h_cur = [None] * S
        t1 = cvp.tile([128, FC], FP32, tag="t1", name="t1")
        t2 = cvp.tile([128, FC], FP32, tag="t2", name="t2")
        # once only one scale remains, its dependent chain is the only
        # work: split it into column halves so two chains pipeline
        # (roughly halves the latency-bound stretch)
        lone_start = max(Kt[s] for s in range(S) if s != last1)
        H = FC // 2
        h_half = [None, None]
        for r in range(max(Kt)):
            for s in range(S):
                if r >= Kt[s]:
                    continue
                if s == last1 and r >= lone_start:
                    for i in range(2):
                        lo, hi = i * H, (i + 1) * H
                        src = (h_half[i][:] if h_half[i] is not None
                               else h_cur[s][:, lo:hi])
                        hh = hp.tile([128, H], FP32, tag=f"hh{i}",
                                     name=f"hh{i}")
                        sth = stp.tile([128, H], FP32, tag=f"sth{i}",
                                       name=f"sth{i}")
                        nc.vector.scalar_tensor_tensor(
                            sth[:], src, whh[s] / wih[s],
                            xcol(s, r, lo, hi), op0=ALU.mult, op1=ALU.add)
                        nc.scalar.activation(hh[:], sth[:], AF.Tanh,
                                             bias=bias_c[:, s:s + 1],
                                             scale=wih[s])
                        h_half[i] = hh
                    continue
                hn = hp.tile([128, FC], FP32, tag=f"h{s}", name=f"h{s}")
                if h_cur[s] is None:
                    nc.scalar.activation(hn[:], xcol(s, r), AF.Tanh,
                                         bias=bias_c[:, s:s + 1],
                                         scale=wih[s])
                else:
                    st = stp.tile([128, FC], FP32, tag=f"st{s}",
                                  name=f"st{s}")
                    nc.vector.scalar_tensor_tensor(
                        st[:], h_cur[s][:], whh[s] / wih[s], xcol(s, r),
                        op0=ALU.mult, op1=ALU.add)
                    nc.scalar.activation(hn[:], st[:], AF.Tanh,
                                         bias=bias_c[:, s:s + 1],
                                         scale=wih[s])
                h_cur[s] = hn
            if r == max(Kt[early2[0]], Kt[early2[1]]) - 1:
                # both early scales done: combine them now (overlaps
                # the remaining lone-scale rounds)
                a, b = early2
                nc.vector.scalar_tensor_tensor(
                    t1[:], h_cur[a][:], c_s[a] / c_s[b], h_cur[b][:],
                    op0=ALU.mult, op1=ALU.add)
        rr = c_s[early2[1]] / c_s[last1]
        if h_half[0] is not None:
            for i in range(2):
                lo, hi = i * H, (i + 1) * H
                nc.vector.scalar_tensor_tensor(
                    t2[:, lo:hi], t1[:, lo:hi], rr, h_half[i][:],
                    op0=ALU.mult, op1=ALU.add)
        else:
            nc.vector.scalar_tensor_tensor(
                t2[:], t1[:], rr, h_cur[last1][:],
                op0=ALU.mult, op1=ALU.add)
        # t2 = sum_s (c_s/c_last) h_s; rnn2 activation scale is c_last
        cpv = c_s[last1]
        u2 = t2[:].rearrange("p (m l) -> p m l", m=Km)

        # ---- rnn2 over m (chain; only last hidden needed) ----
        h2 = None
        feat = smp.tile([128, L2], FP32, tag="feat", name="feat")
        for m in range(Km):
            last = m == Km - 1
            dst = feat[:] if last else r2p.tile(
                [128, L2], FP32, tag="h2", name="h2")[:]
            if h2 is None:
                nc.scalar.activation(dst, u2[:, m, :], AF.Tanh,
                                     bias=bias_c[:, 3:4], scale=cpv)
            else:
                st2 = r2p.tile([128, L2], FP32, tag="st2", name="st2")
                nc.vector.scalar_tensor_tensor(
                    st2[:], h2, whh2 / cpv, u2[:, m, :],
                    op0=ALU.mult, op1=ALU.add)
                nc.scalar.activation(dst, st2[:], AF.Tanh,
                                     bias=bias_c[:, 3:4], scale=cpv)
            h2 = dst

        # ---- BatchNorm stats (partial): sums over local b ----
        featsq = smp.tile([128, L2], FP32, tag="fsq", name="fsq")
        nc.vector.tensor_tensor(featsq[:], feat[:], feat[:], ALU.mult)
        stats = smp.tile([128, 4], FP32, tag="stats", name="stats")
        fv = feat[:].rearrange("p (b eh) -> p eh b", b=BLOC)
        fsv = featsq[:].rearrange("p (b eh) -> p eh b", b=BLOC)
        nc.vector.tensor_reduce(stats[:, 0:2], fv,
                                axis=mybir.AxisListType.X, op=ALU.add)
        nc.vector.tensor_reduce(stats[:, 2:4], fsv,
                                axis=mybir.AxisListType.X, op=ALU.add)

        # AllGather the per-core partial sums (latency floor ~2x lower
        # than AllReduce for this 2KB message).  The stats block is
        # written TRANSPOSED ([4,128]) so the gathered [32,128] result
        # reads back as one contiguous DMA; the transpose-write's small
        # descriptors are hidden before the collective.
        bin_ = dram.tile([4, 128], FP32, tag="bin")
        bout = dram.tile([N_CORES * 4, 128], FP32, tag="bout")
        nc.sync.dma_start(bin_[:].rearrange("f p -> p f"), stats[:])
        nc.gpsimd.collective_compute(
            "AllGather", ALU.bypass,
            replica_groups=[list(range(N_CORES))],
            ins=[bin_.opt()], outs=[bout.opt()])
        stgt = smp.tile([32, 128], FP32, tag="stgt")
        nc.sync.dma_start(stgt[:], bout[:])

        # ---- BN in transposed layout: reduce the 8 blocks via two
        # matmuls -> sums [2,128] and sumsqs [2,128] by e ----
        tailps = pst.tile([128, 512], FP32, tag="tailps")
        psm = tailps[0:2, 128:256]
        psq = tailps[0:2, 320:448]
        nc.tensor.matmul(psm, mred[:, 0:2], stgt[:], start=True, stop=True)
        nc.tensor.matmul(psq, mred[:, 2:4], stgt[:], start=True, stop=True)
        meant = smp.tile([2, 128], FP32, tag="meant")
        nc.vector.tensor_scalar(meant[:], psm, 1.0 / B, None, ALU.mult)
        ex2t = smp.tile([2, 128], FP32, tag="ex2t")
        nc.vector.tensor_scalar(ex2t[:], psq, 1.0 / B, EPS,
                                ALU.mult, op1=ALU.add)
        vart = smp.tile([2, 128], FP32, tag="vart")
        nc.vector.tensor_tensor(vart[:], meant[:], meant[:], ALU.mult)
        nc.vector.tensor_tensor(vart[:], ex2t[:], vart[:], ALU.subtract)
        lnvt = smp.tile([2, 128], FP32, tag="lnvt")
        nc.scalar.activation(lnvt[:], vart[:], AF.Ln)
        istdt = smp.tile([2, 128], FP32, tag="istdt")
        nc.scalar.activation(istdt[:], lnvt[:], AF.Exp, scale=-0.5)
        sclt = smp.tile([2, 128], FP32, tag="sclt")
        nc.vector.tensor_tensor(sclt[:], istdt[:], gamma_t, ALU.mult)
        shft = smp.tile([2, 128], FP32, tag="shft")
        nc.vector.tensor_tensor(shft[:], meant[:], sclt[:], ALU.mult)
        nc.vector.tensor_tensor(shft[:], beta_t, shft[:], ALU.subtract)
        # transpose scl/shf back to per-partition columns [128, 2+2]
        nc.tensor.transpose(tailps[0:128, 256:258], sclt[:],
                            eye4[0:2, 0:2])
        nc.tensor.transpose(tailps[0:128, 258:260], shft[:],
                            eye4[0:2, 0:2])
        ssb = smp.tile([128, 4], FP32, tag="ssb")
        nc.vector.tensor_copy(ssb[:], tailps[0:128, 256:260])

        # ---- normalize + relu fused: relu(scl*f + shf) per e_hi ----
        r = smp.tile([128, L2], FP32, tag="r")
        f3 = feat[:].rearrange("p (b eh) -> p b eh", b=BLOC)
        r3 = r[:].rearrange("p (b eh) -> p b eh", b=BLOC)
        for eh in range(2):
            nc.scalar.activation(r3[:, :, eh], f3[:, :, eh], AF.Relu,
                                 bias=ssb[:, 2 + eh:3 + eh],
                                 scale=ssb[:, eh:eh + 1])

        # ---- FC: logits^T (C, BLOC) = sum_eh Wpack_eh^T @ r[:, :, eh] ----
        pl = tailps[0:C, 0:BLOC]
        nc.tensor.matmul(pl, wpack[:, 0:C], r3[:, :, 0],
                         start=True, stop=False)
        nc.tensor.matmul(pl, wpack[:, C:2 * C], r3[:, :, 1],
                         start=False, stop=True)

        # ---- softmax directly on the (C, BLOC) layout.  Logits are
        # O(10) here (normalized features through W/C weights), so the
        # max-subtraction is unnecessary in fp32.  Sum over classes via
        # a tiny matmul; the host transposes the (C, BLOC) output. ----
        esb = smp.tile([C, BLOC], FP32, tag="esb")
        nc.scalar.activation(esb[:], pl, AF.Exp, bias=fnnb[:, 0:1])
        psum_s = tailps[0:1, 64:64 + BLOC]
        nc.tensor.matmul(psum_s, ones_col, esb[:], start=True, stop=True)
        rin = smp.tile([1, BLOC], FP32, tag="rin")
        nc.vector.reciprocal(rin[:], psum_s)
        pbc = tailps[0:C, 96:96 + BLOC]
        nc.tensor.matmul(pbc, ones_row, rin[:], start=True, stop=True)
        osb = smp.tile([C, BLOC], FP32, tag="osb")
        nc.vector.tensor_tensor(osb[:], esb[:], pbc, ALU.mult)
        nc.sync.dma_start(out=out_dram[:], in_=osb[:])

    nc.compile()
    return nc, Km, Kt, [list(tb) for tb in
                        ([tblocks[s] for s in range(S)])]


def _gather_core(a_list, k, Km, Kt, tblocks):
    """Host-side gather for core k: bf16, layout [e_lo, tblock, t, (m b eh)]."""
    out = []
    for s in range(S):
        T = SCALES[s]
        A = np.asarray(a_list[s])[:, :, 0].reshape(B, M, 2, 128, T)
        Sv = A[k * BLOC:(k + 1) * BLOC, M - Km:, :, :, T - Kt[s]:]
        # [b, m, eh, e_lo, t] -> [e_lo, t, m, b, eh]
        Sv = np.transpose(Sv, (3, 4, 1, 0, 2))
        parts = []
        t_lo = 0
        for tb in tblocks[s]:
            blk = Sv[:, t_lo:t_lo + tb]
            parts.append(np.ascontiguousarray(blk).reshape(128, -1))
            t_lo += tb
        full = np.concatenate(parts, axis=1)
        out.append(full.astype(ml_dtypes.bfloat16).reshape(-1))
    return out


def kernel(a0, a1, a2, rnn1_wih, rnn1_whh, rnn1_bih, rnn1_bhh,
           conv_w, conv_b, rnn2_wih, rnn2_whh, rnn2_bih, rnn2_bhh,
           norm_gamma, norm_beta, fnn_w, fnn_b, _bench=None,
           _km=None, _kt=None):
    params = {
        "wih": [float(rnn1_wih[s]) for s in range(S)],
        "whh": [float(rnn1_whh[s]) for s in range(S)],
        "bb": [float(rnn1_bih[s]) + float(rnn1_bhh[s]) for s in range(S)],
        "cw": [float(conv_w[s]) for s in range(S)],
        "cb": float(conv_b[0]),
        "wih2": float(rnn2_wih[0]),
        "whh2": float(rnn2_whh[0]),
        "bb2": float(rnn2_bih[0]) + float(rnn2_bhh[0]),
        "gamma": np.asarray(norm_gamma, np.float32),
        "beta": np.asarray(norm_beta, np.float32),
        "fnn_w": np.asarray(fnn_w, np.float32),
        "fnn_b": np.asarray(fnn_b, np.float32),
    }
    nc, Km, Kt, tblocks = _build(params, km_override=_km, kt_override=_kt)

    in_maps = []
    for k in range(N_CORES):
        arrs = _gather_core((a0, a1, a2), k, Km, Kt, tblocks)
        in_maps.append({f"a{i}": arrs[i] for i in range(S)})

    kw = dict(_bench) if _bench else {}
    res = run_bass_kernel_spmd(nc, in_maps, core_ids=list(range(N_CORES)),
                               **kw)
    out = np.concatenate([res.results[k]["out"].T for k in range(N_CORES)],
                         axis=0)
    if _bench is not None:
        kernel.last_result = res
    return out
